# revision 11
# baseline (speedup 1.0000x reference)
"""Trainium2 Bass kernel for CSAttention.

Reference computation (per batch b of 32, N=1024 tokens, C=512 channels,
L=512 latent):
    qk  = x @ W_qk.T + b_qk            # [N, 2L]
    q   = qk[:, :L] * L**-0.5
    k   = qk[:, L:]
    out = softmax(q @ k.T, -1) @ y     # [N, C]

Sharding: data-parallel over the batch axis across 8 NeuronCores
(4 batches per core); W_qk / b_qk replicated.

Fast path (b_qk == 0, which setup_inputs always produces):
  logits = q k^T = x (Wq^T Wk) x^T, so precompute M = Wq^T Wk ONCE per
  core (16 matmuls) and per batch do
    zt = M^T-layout @ xt               # [C, N]   (half the flops of qk)
    ST = zt^T x^T  -> exp(scale ST)    # [N, N]   rows m, cols n
    out = ET^T @ [Y | 1] -> normalize  # [N, C]
  eliminating stage B's q/k projection (3.0 -> 2.5 matmul units/batch).
  x is cast f32->bf16 into a DRAM scratch (gpsimd cast DMA) and
  transposed DRAM->SBUF by the DMA xbar (dma_start_transpose), so the
  PE runs zero transposes.

General path (b_qk != 0): baseline 3-stage kernel with explicit q/k.
"""

import numpy as np

import concourse.bass as bass
import concourse.mybir as mybir
import concourse.tile as tile
from concourse import bacc
from concourse.bass_utils import run_bass_kernel_spmd
from concourse.masks import make_identity

P = 128
N_CORES = 8
B_FULL = 32
B_PER_CORE = B_FULL // N_CORES  # 4
N = 1024            # tokens
C = 512             # channels
L = 512             # latent
TWO_L = 2 * L
NT = N // P         # 8 token tiles
CT = C // P         # 4 channel tiles
LT = TWO_L // P     # 8 latent tiles (0..3 = q, 4..7 = k)
SCALE = float(L) ** -0.5
YA = C + 1          # augmented Y width: [Y | ones]
NA = 257            # first stage-D matmul free dim
NB = YA - NA        # 256
SCOL = C - NA       # ones column's index within psum_B (= 255)

F32 = mybir.dt.float32
BF16 = mybir.dt.bfloat16
IDENT = mybir.ActivationFunctionType.Identity
EXP = mybir.ActivationFunctionType.Exp


def _emit_fast(tc, x, y, w, out, xbf_dram):
    """Fast path (no bias). x/y: [B_PER_CORE, N, C] dram APs, w: [2L, C],
    out: [B_PER_CORE, N, C], xbf_dram: [2, N, C] bf16 dram scratch for the
    xbar-transposed loads of batches 1..3."""
    from contextlib import ExitStack

    nc = tc.nc
    with ExitStack() as ctx:
        const = ctx.enter_context(tc.tile_pool(name="const", bufs=1))
        big = ctx.enter_context(tc.tile_pool(name="big", bufs=1))
        outp = ctx.enter_context(tc.tile_pool(name="outp", bufs=4))
        rsp = ctx.enter_context(tc.tile_pool(name="rsp", bufs=4))
        ps_mm = ctx.enter_context(tc.tile_pool(name="ps_mm", bufs=3, space="PSUM"))
        ps_d = ctx.enter_context(tc.tile_pool(name="ps_d", bufs=3, space="PSUM"))
        ps_tr = ctx.enter_context(tc.tile_pool(name="ps_tr", bufs=2, space="PSUM"))

        identity = const.tile([P, P], BF16, tag="ident")
        make_identity(nc, identity)

        # ---- persistent workspaces ----
        wbf = big.tile([P, LT, C], BF16, tag="wbf")      # W natural [l, c]
        mt_sb = big.tile([P, CT, C], BF16, tag="mt")     # MT[b, a] = M[a, b]
        zt = big.tile([P, CT, N], BF16, tag="zt")        # [c', n]
        et = big.tile([P, NT, N], BF16, tag="et")        # rows m, cols n
        xbf0 = big.tile([P, NT, C], BF16, tag="xbf0")    # batch-0 staging
        # xt[p, h, cb, nn] = x[h*512+nn, cb*128+p]; each half contiguous so
        # it can be an xbar-transpose destination.
        xt2 = [
            big.tile([P, 2, CT, 512], BF16, tag=f"xt{j}", name=f"xt{j}")
            for j in range(2)
        ]
        yaug2 = [
            big.tile([P, NT, YA], BF16, tag=f"yaug{j}", name=f"yaug{j}")
            for j in range(2)
        ]
        for j in range(2):
            nc.vector.memset(yaug2[j][:, :, C:YA], 1.0)

        w_tiled = w.rearrange("(t p) c -> p t c", p=P)

        def cast_x_dram(i):
            # f32 dram -> bf16 dram scratch (gpsimd casting DMA), 2 halves
            buf = xbf_dram[i % 2]
            for h in range(2):
                nc.gpsimd.dma_start(
                    buf[h * 512:(h + 1) * 512, :], x[i, h * 512:(h + 1) * 512, :]
                )

        def transpose_x0(lo, hi):
            for nt_ in range(lo, hi):
                h, r = nt_ // 4, nt_ % 4
                for ct in range(CT):
                    ps = ps_tr.tile([P, P], BF16, tag="tr")
                    nc.tensor.transpose(
                        ps, xbf0[:, nt_, ct * P:(ct + 1) * P], identity
                    )
                    nc.vector.tensor_copy(
                        xt2[0][:, h, ct, r * P:(r + 1) * P], ps
                    )

        def load_y(i):
            nc.gpsimd.dma_start(
                yaug2[i % 2][:, :, 0:C], y[i].rearrange("(t p) c -> p t c", p=P)
            )

        def emit_zt(xt, nh):
            # zt[a, n] = sum_b MT[b, a] xt[b, n]
            for ca in range(CT):
                ps = ps_mm.tile([P, 512], F32, tag="mm")
                for cb in range(CT):
                    nc.tensor.matmul(
                        ps,
                        mt_sb[:, cb, ca * P:(ca + 1) * P],
                        xt[:, nh, cb, :],
                        start=(cb == 0),
                        stop=(cb == CT - 1),
                    )
                nc.vector.tensor_copy(zt[:, ca, nh * 512:(nh + 1) * 512], ps)

        # ---- startup: x0 + W interleaved on the gpsimd cast-DMA queue ----
        # (All DMA queues share a ~9us spin-up before the first byte lands;
        # interleaving x0 chunks with W halves minimizes time-to-first-ST.)
        x0_tiled = x[0].rearrange("(t p) c -> p t c", p=P)
        nc.gpsimd.dma_start(xbf0[:, 0:2], x0_tiled[:, 0:2])
        nc.gpsimd.dma_start(wbf[:, 0:4], w_tiled[:, 0:4])   # Wq
        nc.gpsimd.dma_start(xbf0[:, 2:4], x0_tiled[:, 2:4])
        nc.gpsimd.dma_start(wbf[:, 4:8], w_tiled[:, 4:8])   # Wk
        nc.gpsimd.dma_start(xbf0[:, 4:6], x0_tiled[:, 4:6])
        nc.gpsimd.dma_start(xbf0[:, 6:8], x0_tiled[:, 6:8])
        load_y(0)

        transpose_x0(0, 4)

        # MT[b, a] = M[a, b] = sum_l Wq[l, a] Wk[l, b]
        for cb in range(CT):
            ps = ps_mm.tile([P, C], F32, tag="mm")
            for lt in range(4):
                nc.tensor.matmul(
                    ps,
                    wbf[:, 4 + lt, cb * P:(cb + 1) * P],
                    wbf[:, lt, 0:C],
                    start=(lt == 0),
                    stop=(lt == 3),
                )
            nc.vector.tensor_copy(mt_sb[:, cb, :], ps)

        transpose_x0(4, NT)

        for i in range(B_PER_CORE):
            if i + 1 < B_PER_CORE:
                cast_x_dram(i + 1)
                load_y(i + 1)
            xt = xt2[i % 2]
            xt_next = xt2[(i + 1) % 2]
            yaug = yaug2[i % 2]

            emit_zt(xt, 0)
            emit_zt(xt, 1)

            # xbar-transpose next batch's x half 0 (sync queue; issued here
            # so the wait for the cast DMA blocks only the idle sync engine)
            if i + 1 < B_PER_CORE:
                nc.sync.dma_start_transpose(
                    xt_next[:, 0], xbf_dram[(i + 1) % 2][0:512, :]
                )

            # ---- ST[m, n] = sum_a zt[a, m] xt[a, n]; ET = exp(scale ST) ----
            for nh in range(2):
                for mt in range(NT):
                    ps = ps_mm.tile([P, 512], F32, tag="mm")
                    for ca in range(CT):
                        nc.tensor.matmul(
                            ps,
                            zt[:, ca, mt * P:(mt + 1) * P],
                            xt[:, nh, ca, :],
                            start=(ca == 0),
                            stop=(ca == CT - 1),
                        )
                    nc.scalar.activation(
                        et[:, mt, nh * 512:(nh + 1) * 512], ps, EXP, scale=SCALE
                    )

            # xbar-transpose next batch's x half 1 (scalar queue; by now the
            # cast DMA is long done so this does not stall the ACT stream)
            if i + 1 < B_PER_CORE:
                nc.scalar.dma_start_transpose(
                    xt_next[:, 1], xbf_dram[(i + 1) % 2][512:1024, :]
                )

            # ---- stage D: out = ET.T @ [Y | 1], then normalize ----
            for nt_ in range(NT):
                psA = ps_d.tile([P, NA], F32, tag="d")
                psB = ps_d.tile([P, NB], F32, tag="d")
                for mt in range(NT):
                    lw = et[:, mt, nt_ * P:(nt_ + 1) * P]
                    nc.tensor.matmul(
                        psA, lw, yaug[:, mt, 0:NA],
                        start=(mt == 0), stop=(mt == NT - 1),
                    )
                    nc.tensor.matmul(
                        psB, lw, yaug[:, mt, NA:YA],
                        start=(mt == 0), stop=(mt == NT - 1),
                    )
                rs = rsp.tile([P, 1], F32, tag="rs")
                nc.vector.reciprocal(rs, psB[:, SCOL:SCOL + 1])
                ob = outp.tile([P, C], F32, tag="ob")
                nc.scalar.mul(ob[:, 0:NA], psA[:, 0:NA], rs)
                nc.vector.tensor_scalar_mul(ob[:, NA:C], psB[:, 0:SCOL], rs)
                if nt_ % 2 == 0:
                    nc.sync.dma_start(out[i, nt_ * P:(nt_ + 1) * P, :], ob)
                else:
                    nc.scalar.dma_start(out[i, nt_ * P:(nt_ + 1) * P, :], ob)


def _emit_general(tc, x, y, w, bvec, out):
    """General path (nonzero bias): baseline 3-stage kernel."""
    from contextlib import ExitStack

    nc = tc.nc
    with ExitStack() as ctx:
        const = ctx.enter_context(tc.tile_pool(name="const", bufs=1))
        big = ctx.enter_context(tc.tile_pool(name="big", bufs=1))
        outp = ctx.enter_context(tc.tile_pool(name="outp", bufs=4))
        rsp = ctx.enter_context(tc.tile_pool(name="rsp", bufs=4))
        ps_mm = ctx.enter_context(tc.tile_pool(name="ps_mm", bufs=3, space="PSUM"))
        ps_d = ctx.enter_context(tc.tile_pool(name="ps_d", bufs=3, space="PSUM"))
        ps_tr = ctx.enter_context(tc.tile_pool(name="ps_tr", bufs=2, space="PSUM"))

        bias_sb = const.tile([P, LT], F32, tag="bias")
        nc.sync.dma_start(bias_sb, bvec.rearrange("(o p) -> p o", p=P))

        wbf = big.tile([P, LT, C], BF16, tag="wbf")
        w_tiled = w.rearrange("(t p) c -> p t c", p=P)
        wt = big.tile([P, CT, TWO_L], BF16, tag="wt")

        identity = const.tile([P, P], BF16, tag="ident")
        make_identity(nc, identity)

        xbf2 = [
            big.tile([P, NT, C], BF16, tag=f"xbf{j}", name=f"xbf{j}")
            for j in range(2)
        ]
        xt2 = [
            big.tile([P, CT, N], BF16, tag=f"xt{j}", name=f"xt{j}")
            for j in range(2)
        ]
        qkt = big.tile([P, LT, N], BF16, tag="qkt")
        et = big.tile([P, NT, N], BF16, tag="et")
        yaug = big.tile([P, NT, YA], BF16, tag="yaug")
        nc.vector.memset(yaug[:, :, C:YA], 1.0)

        def load_x(i):
            x_tiled = x[i].rearrange("(t p) c -> p t c", p=P)
            for j in range(2):
                nc.gpsimd.dma_start(
                    xbf2[i % 2][:, 4 * j:4 * j + 4], x_tiled[:, 4 * j:4 * j + 4]
                )

        def transpose_x(i):
            src, dst = xbf2[i % 2], xt2[i % 2]
            for nt_ in range(NT):
                for ct in range(CT):
                    ps = ps_tr.tile([P, P], BF16, tag="tr")
                    nc.tensor.transpose(
                        ps, src[:, nt_, ct * P:(ct + 1) * P], identity
                    )
                    nc.vector.tensor_copy(dst[:, ct, nt_ * P:(nt_ + 1) * P], ps)

        def load_y(i):
            nc.gpsimd.dma_start(
                yaug[:, :, 0:C], y[i].rearrange("(t p) c -> p t c", p=P)
            )

        x0_tiled = x[0].rearrange("(t p) c -> p t c", p=P)
        nc.gpsimd.dma_start(xbf2[0][:, 0:4], x0_tiled[:, 0:4])
        for a, b in ((0, 1), (1, 2), (2, 4), (4, 8)):
            nc.gpsimd.dma_start(wbf[:, a:b], w_tiled[:, a:b])
        nc.gpsimd.dma_start(xbf2[0][:, 4:8], x0_tiled[:, 4:8])
        for nt_ in range(4):
            for ct in range(CT):
                ps = ps_tr.tile([P, P], BF16, tag="tr")
                nc.tensor.transpose(ps, xbf2[0][:, nt_, ct * P:(ct + 1) * P], identity)
                nc.vector.tensor_copy(xt2[0][:, ct, nt_ * P:(nt_ + 1) * P], ps)
        for lt in range(LT):
            for ct in range(CT):
                ps = ps_tr.tile([P, P], BF16, tag="tr")
                nc.tensor.transpose(
                    ps, wbf[:, lt, ct * P:(ct + 1) * P], identity
                )
                nc.vector.tensor_copy(wt[:, ct, lt * P:(lt + 1) * P], ps)
        for nt_ in range(4, NT):
            for ct in range(CT):
                ps = ps_tr.tile([P, P], BF16, tag="tr")
                nc.tensor.transpose(ps, xbf2[0][:, nt_, ct * P:(ct + 1) * P], identity)
                nc.vector.tensor_copy(xt2[0][:, ct, nt_ * P:(nt_ + 1) * P], ps)

        for i in range(B_PER_CORE):
            if i + 1 < B_PER_CORE:
                load_x(i + 1)
            load_y(i)
            xt = xt2[i % 2]

            for nh in range(2):
                for lt in range(LT):
                    ps = ps_mm.tile([P, 512], F32, tag="mm")
                    for ct in range(CT):
                        nc.tensor.matmul(
                            ps,
                            wt[:, ct, lt * P:(lt + 1) * P],
                            xt[:, ct, nh * 512:(nh + 1) * 512],
                            start=(ct == 0),
                            stop=(ct == CT - 1),
                        )
                    nc.scalar.activation(
                        qkt[:, lt, nh * 512:(nh + 1) * 512],
                        ps,
                        IDENT,
                        bias=bias_sb[:, lt:lt + 1],
                    )

            for nh in range(2):
                for mt in range(NT):
                    ps = ps_mm.tile([P, 512], F32, tag="mm")
                    for lq in range(4):
                        nc.tensor.matmul(
                            ps,
                            qkt[:, 4 + lq, mt * P:(mt + 1) * P],
                            qkt[:, lq, nh * 512:(nh + 1) * 512],
                            start=(lq == 0),
                            stop=(lq == 3),
                        )
                    nc.scalar.activation(
                        et[:, mt, nh * 512:(nh + 1) * 512], ps, EXP, scale=SCALE
                    )

            if i + 1 < B_PER_CORE:
                transpose_x(i + 1)

            for nt_ in range(NT):
                psA = ps_d.tile([P, NA], F32, tag="d")
                psB = ps_d.tile([P, NA], F32, tag="d")
                for mt in range(NT):
                    lw = et[:, mt, nt_ * P:(nt_ + 1) * P]
                    nc.tensor.matmul(
                        psA, lw, yaug[:, mt, 0:NA],
                        start=(mt == 0), stop=(mt == NT - 1),
                    )
                    nc.tensor.matmul(
                        psB[:, 0:NB], lw, yaug[:, mt, NA:YA],
                        start=(mt == 0), stop=(mt == NT - 1),
                    )
                rs = rsp.tile([P, 1], F32, tag="rs")
                nc.vector.reciprocal(rs, psB[:, SCOL:SCOL + 1])
                ob = outp.tile([P, C], F32, tag="ob")
                nc.scalar.mul(ob[:, 0:NA], psA[:, 0:NA], rs)
                nc.vector.tensor_scalar_mul(ob[:, NA:C], psB[:, 0:SCOL], rs)
                nc.sync.dma_start(out[i, nt_ * P:(nt_ + 1) * P, :], ob)


_NC_CACHE = {}


def _build(fast):
    key = ("fast" if fast else "general")
    if key in _NC_CACHE:
        return _NC_CACHE[key]
    nc = bacc.Bacc(
        "TRN2",
        target_bir_lowering=False,
        debug=False,
        enable_asserts=False,
        num_devices=N_CORES,
    )
    x = nc.dram_tensor("x", [B_PER_CORE, N, C], F32, kind="ExternalInput").ap()
    y = nc.dram_tensor("y", [B_PER_CORE, N, C], F32, kind="ExternalInput").ap()
    w = nc.dram_tensor("W_qk", [TWO_L, C], F32, kind="ExternalInput").ap()
    bvec = nc.dram_tensor("b_qk", [TWO_L], F32, kind="ExternalInput").ap()
    out = nc.dram_tensor("out", [B_PER_CORE, N, C], F32, kind="ExternalOutput").ap()
    if fast:
        xbf_dram = nc.dram_tensor("xbf_scratch", [2, N, C], BF16, kind="Internal").ap()
        with tile.TileContext(nc) as tc:
            _emit_fast(tc, x, y, w, out, xbf_dram)
    else:
        with tile.TileContext(nc) as tc:
            _emit_general(tc, x, y, w, bvec, out)
    nc.compile()
    _NC_CACHE[key] = nc
    return nc


def run(x, y, W_qk, b_qk, trace=False):
    """Run the SPMD kernel on 8 cores; returns (out, BassKernelResults)."""
    x = np.ascontiguousarray(x, dtype=np.float32)
    y = np.ascontiguousarray(y, dtype=np.float32)
    W_qk = np.ascontiguousarray(W_qk, dtype=np.float32)
    b_qk = np.ascontiguousarray(b_qk, dtype=np.float32)
    fast = not np.any(b_qk)
    nc = _build(fast)
    in_maps = [
        {
            "x": x[k * B_PER_CORE:(k + 1) * B_PER_CORE],
            "y": y[k * B_PER_CORE:(k + 1) * B_PER_CORE],
            "W_qk": W_qk,
            "b_qk": b_qk,
        }
        for k in range(N_CORES)
    ]
    res = run_bass_kernel_spmd(
        nc, in_maps, core_ids=list(range(N_CORES)), trace=trace
    )
    outs = [r["out"] for r in res.results]
    return np.concatenate(outs, axis=0), res


def kernel(x, y, W_qk, b_qk):
    out, _ = run(x, y, W_qk, b_qk)
    return out


# revision 14
# speedup vs baseline: 1.0569x; 1.0569x over previous
"""Trainium2 Bass kernel for CSAttention.

Reference computation (per batch b of 32, N=1024 tokens, C=512 channels,
L=512 latent):
    qk  = x @ W_qk.T + b_qk            # [N, 2L]
    q   = qk[:, :L] * L**-0.5
    k   = qk[:, L:]
    out = softmax(q @ k.T, -1) @ y     # [N, C]

Sharding: data-parallel over the batch axis across 8 NeuronCores
(4 batches per core); W_qk / b_qk replicated.

Fast path (b_qk == 0, which setup_inputs always produces):
  logits = q k^T = x (Wq^T Wk) x^T, so precompute M = Wq^T Wk ONCE per
  core (16 matmuls) and per batch do
    zt = M^T-layout @ xt               # [C, N]   (half the flops of qk)
    ST = zt^T x^T  -> exp(scale ST)    # [N, N]   rows m, cols n
    out = ET^T @ [Y | 1] -> normalize  # [N, C]
  eliminating stage B's q/k projection (3.0 -> 2.5 matmul units/batch).
  x is cast f32->bf16 into a DRAM scratch (gpsimd cast DMA) and
  transposed DRAM->SBUF by the DMA xbar (dma_start_transpose), so the
  PE runs zero transposes.

General path (b_qk != 0): baseline 3-stage kernel with explicit q/k.
"""

import numpy as np

import concourse.bass as bass
import concourse.mybir as mybir
import concourse.tile as tile
from concourse import bacc
from concourse.bass_utils import run_bass_kernel_spmd
from concourse.masks import make_identity

P = 128
N_CORES = 8
B_FULL = 32
B_PER_CORE = B_FULL // N_CORES  # 4
N = 1024            # tokens
C = 512             # channels
L = 512             # latent
TWO_L = 2 * L
NT = N // P         # 8 token tiles
CT = C // P         # 4 channel tiles
LT = TWO_L // P     # 8 latent tiles (0..3 = q, 4..7 = k)
SCALE = float(L) ** -0.5
YA = C + 1          # augmented Y width: [Y | ones]
NA = 257            # first stage-D matmul free dim
NB = YA - NA        # 256
SCOL = C - NA       # ones column's index within psum_B (= 255)

F32 = mybir.dt.float32
BF16 = mybir.dt.bfloat16
IDENT = mybir.ActivationFunctionType.Identity
EXP = mybir.ActivationFunctionType.Exp


def _emit_fast(tc, x, y, w, out):
    """Fast path (no bias). x/y: [B_PER_CORE, N, C] dram APs, w: [2L, C],
    out: [B_PER_CORE, N, C]."""
    from contextlib import ExitStack

    nc = tc.nc
    with ExitStack() as ctx:
        const = ctx.enter_context(tc.tile_pool(name="const", bufs=1))
        big = ctx.enter_context(tc.tile_pool(name="big", bufs=1))
        outp = ctx.enter_context(tc.tile_pool(name="outp", bufs=4))
        rsp = ctx.enter_context(tc.tile_pool(name="rsp", bufs=4))
        ps_mm = ctx.enter_context(tc.tile_pool(name="ps_mm", bufs=3, space="PSUM"))
        ps_d = ctx.enter_context(tc.tile_pool(name="ps_d", bufs=3, space="PSUM"))
        ps_tr = ctx.enter_context(tc.tile_pool(name="ps_tr", bufs=2, space="PSUM"))

        identity = const.tile([P, P], BF16, tag="ident")
        make_identity(nc, identity)

        # ---- persistent workspaces ----
        wbf = big.tile([P, LT, C], BF16, tag="wbf")      # W natural [l, c]
        mt_sb = big.tile([P, CT, C], BF16, tag="mt")     # MT[b, a] = M[a, b]
        zt = big.tile([P, CT, N], BF16, tag="zt")        # [c', n]
        et = big.tile([P, NT, N], BF16, tag="et")        # rows m, cols n
        xbf2 = [
            big.tile([P, NT, C], BF16, tag=f"xbf{j}", name=f"xbf{j}")
            for j in range(2)
        ]
        xt2 = [
            big.tile([P, CT, N], BF16, tag=f"xt{j}", name=f"xt{j}")
            for j in range(2)
        ]
        yaug2 = [
            big.tile([P, NT, YA], BF16, tag=f"yaug{j}", name=f"yaug{j}")
            for j in range(2)
        ]
        for j in range(2):
            nc.vector.memset(yaug2[j][:, :, C:YA], 1.0)

        w_tiled = w.rearrange("(t p) c -> p t c", p=P)

        def load_x(i):
            x_tiled = x[i].rearrange("(t p) c -> p t c", p=P)
            for j in range(2):
                nc.gpsimd.dma_start(
                    xbf2[i % 2][:, 4 * j:4 * j + 4], x_tiled[:, 4 * j:4 * j + 4]
                )

        def transpose_x(i, lo=0, hi=NT):
            src, dst = xbf2[i % 2], xt2[i % 2]
            for nt_ in range(lo, hi):
                for ct in range(CT):
                    ps = ps_tr.tile([P, P], BF16, tag="tr")
                    nc.tensor.transpose(
                        ps, src[:, nt_, ct * P:(ct + 1) * P], identity
                    )
                    nc.vector.tensor_copy(dst[:, ct, nt_ * P:(nt_ + 1) * P], ps)

        def load_y(i):
            nc.gpsimd.dma_start(
                yaug2[i % 2][:, :, 0:C], y[i].rearrange("(t p) c -> p t c", p=P)
            )

        def emit_zt(xt, nh):
            # zt[a, n] = sum_b MT[b, a] xt[b, n]
            for ca in range(CT):
                ps = ps_mm.tile([P, 512], F32, tag="mm")
                for cb in range(CT):
                    nc.tensor.matmul(
                        ps,
                        mt_sb[:, cb, ca * P:(ca + 1) * P],
                        xt[:, cb, nh * 512:(nh + 1) * 512],
                        start=(cb == 0),
                        stop=(cb == CT - 1),
                    )
                nc.vector.tensor_copy(zt[:, ca, nh * 512:(nh + 1) * 512], ps)

        # ---- startup: x0 + W interleaved on the gpsimd cast-DMA queue ----
        # (All DMA queues share a ~9us spin-up before the first byte lands;
        # interleaving x0 chunks with W halves minimizes time-to-first-ST.)
        x0_tiled = x[0].rearrange("(t p) c -> p t c", p=P)
        nc.gpsimd.dma_start(xbf2[0][:, 0:2], x0_tiled[:, 0:2])
        nc.gpsimd.dma_start(wbf[:, 0:4], w_tiled[:, 0:4])   # Wq
        nc.gpsimd.dma_start(xbf2[0][:, 2:4], x0_tiled[:, 2:4])
        nc.gpsimd.dma_start(wbf[:, 4:8], w_tiled[:, 4:8])   # Wk
        nc.gpsimd.dma_start(xbf2[0][:, 4:6], x0_tiled[:, 4:6])
        nc.gpsimd.dma_start(xbf2[0][:, 6:8], x0_tiled[:, 6:8])
        load_y(0)

        transpose_x(0, 0, 4)

        # MT[b, a] = M[a, b] = sum_l Wq[l, a] Wk[l, b]
        for cb in range(CT):
            ps = ps_mm.tile([P, C], F32, tag="mm")
            for lt in range(4):
                nc.tensor.matmul(
                    ps,
                    wbf[:, 4 + lt, cb * P:(cb + 1) * P],
                    wbf[:, lt, 0:C],
                    start=(lt == 0),
                    stop=(lt == 3),
                )
            nc.vector.tensor_copy(mt_sb[:, cb, :], ps)

        transpose_x(0, 4, NT)

        for i in range(B_PER_CORE):
            if i + 1 < B_PER_CORE:
                load_x(i + 1)
                load_y(i + 1)
            xt = xt2[i % 2]
            yaug = yaug2[i % 2]

            emit_zt(xt, 0)
            emit_zt(xt, 1)

            # ---- ST[m, n] = sum_a zt[a, m] xt[a, n]; ET = exp(scale ST) ----
            for nh in range(2):
                for mt in range(NT):
                    ps = ps_mm.tile([P, 512], F32, tag="mm")
                    for ca in range(CT):
                        nc.tensor.matmul(
                            ps,
                            zt[:, ca, mt * P:(mt + 1) * P],
                            xt[:, ca, nh * 512:(nh + 1) * 512],
                            start=(ca == 0),
                            stop=(ca == CT - 1),
                        )
                    nc.scalar.activation(
                        et[:, mt, nh * 512:(nh + 1) * 512], ps, EXP, scale=SCALE
                    )

            # transpose next batch's x on the PE (between C and D)
            if i + 1 < B_PER_CORE:
                transpose_x(i + 1)

            # ---- stage D: out = ET.T @ [Y | 1], then normalize ----
            for nt_ in range(NT):
                psA = ps_d.tile([P, NA], F32, tag="d")
                psB = ps_d.tile([P, NB], F32, tag="d")
                for mt in range(NT):
                    lw = et[:, mt, nt_ * P:(nt_ + 1) * P]
                    nc.tensor.matmul(
                        psA, lw, yaug[:, mt, 0:NA],
                        start=(mt == 0), stop=(mt == NT - 1),
                    )
                    nc.tensor.matmul(
                        psB, lw, yaug[:, mt, NA:YA],
                        start=(mt == 0), stop=(mt == NT - 1),
                    )
                rs = rsp.tile([P, 1], F32, tag="rs")
                nc.vector.reciprocal(rs, psB[:, SCOL:SCOL + 1])
                ob = outp.tile([P, C], F32, tag="ob")
                nc.scalar.mul(ob[:, 0:NA], psA[:, 0:NA], rs)
                nc.vector.tensor_scalar_mul(ob[:, NA:C], psB[:, 0:SCOL], rs)
                if nt_ % 2 == 0:
                    nc.sync.dma_start(out[i, nt_ * P:(nt_ + 1) * P, :], ob)
                else:
                    nc.scalar.dma_start(out[i, nt_ * P:(nt_ + 1) * P, :], ob)


def _emit_general(tc, x, y, w, bvec, out):
    """General path (nonzero bias): baseline 3-stage kernel."""
    from contextlib import ExitStack

    nc = tc.nc
    with ExitStack() as ctx:
        const = ctx.enter_context(tc.tile_pool(name="const", bufs=1))
        big = ctx.enter_context(tc.tile_pool(name="big", bufs=1))
        outp = ctx.enter_context(tc.tile_pool(name="outp", bufs=4))
        rsp = ctx.enter_context(tc.tile_pool(name="rsp", bufs=4))
        ps_mm = ctx.enter_context(tc.tile_pool(name="ps_mm", bufs=3, space="PSUM"))
        ps_d = ctx.enter_context(tc.tile_pool(name="ps_d", bufs=3, space="PSUM"))
        ps_tr = ctx.enter_context(tc.tile_pool(name="ps_tr", bufs=2, space="PSUM"))

        bias_sb = const.tile([P, LT], F32, tag="bias")
        nc.sync.dma_start(bias_sb, bvec.rearrange("(o p) -> p o", p=P))

        wbf = big.tile([P, LT, C], BF16, tag="wbf")
        w_tiled = w.rearrange("(t p) c -> p t c", p=P)
        wt = big.tile([P, CT, TWO_L], BF16, tag="wt")

        identity = const.tile([P, P], BF16, tag="ident")
        make_identity(nc, identity)

        xbf2 = [
            big.tile([P, NT, C], BF16, tag=f"xbf{j}", name=f"xbf{j}")
            for j in range(2)
        ]
        xt2 = [
            big.tile([P, CT, N], BF16, tag=f"xt{j}", name=f"xt{j}")
            for j in range(2)
        ]
        qkt = big.tile([P, LT, N], BF16, tag="qkt")
        et = big.tile([P, NT, N], BF16, tag="et")
        yaug = big.tile([P, NT, YA], BF16, tag="yaug")
        nc.vector.memset(yaug[:, :, C:YA], 1.0)

        def load_x(i):
            x_tiled = x[i].rearrange("(t p) c -> p t c", p=P)
            for j in range(2):
                nc.gpsimd.dma_start(
                    xbf2[i % 2][:, 4 * j:4 * j + 4], x_tiled[:, 4 * j:4 * j + 4]
                )

        def transpose_x(i):
            src, dst = xbf2[i % 2], xt2[i % 2]
            for nt_ in range(NT):
                for ct in range(CT):
                    ps = ps_tr.tile([P, P], BF16, tag="tr")
                    nc.tensor.transpose(
                        ps, src[:, nt_, ct * P:(ct + 1) * P], identity
                    )
                    nc.vector.tensor_copy(dst[:, ct, nt_ * P:(nt_ + 1) * P], ps)

        def load_y(i):
            nc.gpsimd.dma_start(
                yaug[:, :, 0:C], y[i].rearrange("(t p) c -> p t c", p=P)
            )

        x0_tiled = x[0].rearrange("(t p) c -> p t c", p=P)
        nc.gpsimd.dma_start(xbf2[0][:, 0:4], x0_tiled[:, 0:4])
        for a, b in ((0, 1), (1, 2), (2, 4), (4, 8)):
            nc.gpsimd.dma_start(wbf[:, a:b], w_tiled[:, a:b])
        nc.gpsimd.dma_start(xbf2[0][:, 4:8], x0_tiled[:, 4:8])
        for nt_ in range(4):
            for ct in range(CT):
                ps = ps_tr.tile([P, P], BF16, tag="tr")
                nc.tensor.transpose(ps, xbf2[0][:, nt_, ct * P:(ct + 1) * P], identity)
                nc.vector.tensor_copy(xt2[0][:, ct, nt_ * P:(nt_ + 1) * P], ps)
        for lt in range(LT):
            for ct in range(CT):
                ps = ps_tr.tile([P, P], BF16, tag="tr")
                nc.tensor.transpose(
                    ps, wbf[:, lt, ct * P:(ct + 1) * P], identity
                )
                nc.vector.tensor_copy(wt[:, ct, lt * P:(lt + 1) * P], ps)
        for nt_ in range(4, NT):
            for ct in range(CT):
                ps = ps_tr.tile([P, P], BF16, tag="tr")
                nc.tensor.transpose(ps, xbf2[0][:, nt_, ct * P:(ct + 1) * P], identity)
                nc.vector.tensor_copy(xt2[0][:, ct, nt_ * P:(nt_ + 1) * P], ps)

        for i in range(B_PER_CORE):
            if i + 1 < B_PER_CORE:
                load_x(i + 1)
            load_y(i)
            xt = xt2[i % 2]

            for nh in range(2):
                for lt in range(LT):
                    ps = ps_mm.tile([P, 512], F32, tag="mm")
                    for ct in range(CT):
                        nc.tensor.matmul(
                            ps,
                            wt[:, ct, lt * P:(lt + 1) * P],
                            xt[:, ct, nh * 512:(nh + 1) * 512],
                            start=(ct == 0),
                            stop=(ct == CT - 1),
                        )
                    nc.scalar.activation(
                        qkt[:, lt, nh * 512:(nh + 1) * 512],
                        ps,
                        IDENT,
                        bias=bias_sb[:, lt:lt + 1],
                    )

            for nh in range(2):
                for mt in range(NT):
                    ps = ps_mm.tile([P, 512], F32, tag="mm")
                    for lq in range(4):
                        nc.tensor.matmul(
                            ps,
                            qkt[:, 4 + lq, mt * P:(mt + 1) * P],
                            qkt[:, lq, nh * 512:(nh + 1) * 512],
                            start=(lq == 0),
                            stop=(lq == 3),
                        )
                    nc.scalar.activation(
                        et[:, mt, nh * 512:(nh + 1) * 512], ps, EXP, scale=SCALE
                    )

            if i + 1 < B_PER_CORE:
                transpose_x(i + 1)

            for nt_ in range(NT):
                psA = ps_d.tile([P, NA], F32, tag="d")
                psB = ps_d.tile([P, NA], F32, tag="d")
                for mt in range(NT):
                    lw = et[:, mt, nt_ * P:(nt_ + 1) * P]
                    nc.tensor.matmul(
                        psA, lw, yaug[:, mt, 0:NA],
                        start=(mt == 0), stop=(mt == NT - 1),
                    )
                    nc.tensor.matmul(
                        psB[:, 0:NB], lw, yaug[:, mt, NA:YA],
                        start=(mt == 0), stop=(mt == NT - 1),
                    )
                rs = rsp.tile([P, 1], F32, tag="rs")
                nc.vector.reciprocal(rs, psB[:, SCOL:SCOL + 1])
                ob = outp.tile([P, C], F32, tag="ob")
                nc.scalar.mul(ob[:, 0:NA], psA[:, 0:NA], rs)
                nc.vector.tensor_scalar_mul(ob[:, NA:C], psB[:, 0:SCOL], rs)
                nc.sync.dma_start(out[i, nt_ * P:(nt_ + 1) * P, :], ob)


_NC_CACHE = {}


def _build(fast):
    key = ("fast" if fast else "general")
    if key in _NC_CACHE:
        return _NC_CACHE[key]
    nc = bacc.Bacc(
        "TRN2",
        target_bir_lowering=False,
        debug=False,
        enable_asserts=False,
        num_devices=N_CORES,
    )
    x = nc.dram_tensor("x", [B_PER_CORE, N, C], F32, kind="ExternalInput").ap()
    y = nc.dram_tensor("y", [B_PER_CORE, N, C], F32, kind="ExternalInput").ap()
    w = nc.dram_tensor("W_qk", [TWO_L, C], F32, kind="ExternalInput").ap()
    bvec = nc.dram_tensor("b_qk", [TWO_L], F32, kind="ExternalInput").ap()
    out = nc.dram_tensor("out", [B_PER_CORE, N, C], F32, kind="ExternalOutput").ap()
    if fast:
        with tile.TileContext(nc) as tc:
            _emit_fast(tc, x, y, w, out)
    else:
        with tile.TileContext(nc) as tc:
            _emit_general(tc, x, y, w, bvec, out)
    nc.compile()
    _NC_CACHE[key] = nc
    return nc


def run(x, y, W_qk, b_qk, trace=False):
    """Run the SPMD kernel on 8 cores; returns (out, BassKernelResults)."""
    x = np.ascontiguousarray(x, dtype=np.float32)
    y = np.ascontiguousarray(y, dtype=np.float32)
    W_qk = np.ascontiguousarray(W_qk, dtype=np.float32)
    b_qk = np.ascontiguousarray(b_qk, dtype=np.float32)
    fast = not np.any(b_qk)
    nc = _build(fast)
    in_maps = [
        {
            "x": x[k * B_PER_CORE:(k + 1) * B_PER_CORE],
            "y": y[k * B_PER_CORE:(k + 1) * B_PER_CORE],
            "W_qk": W_qk,
            "b_qk": b_qk,
        }
        for k in range(N_CORES)
    ]
    res = run_bass_kernel_spmd(
        nc, in_maps, core_ids=list(range(N_CORES)), trace=trace
    )
    outs = [r["out"] for r in res.results]
    return np.concatenate(outs, axis=0), res


def kernel(x, y, W_qk, b_qk):
    out, _ = run(x, y, W_qk, b_qk)
    return out


# revision 15
# speedup vs baseline: 1.0837x; 1.0253x over previous
"""Trainium2 Bass kernel for CSAttention.

Reference computation (per batch b of 32, N=1024 tokens, C=512 channels,
L=512 latent):
    qk  = x @ W_qk.T + b_qk            # [N, 2L]
    q   = qk[:, :L] * L**-0.5
    k   = qk[:, L:]
    out = softmax(q @ k.T, -1) @ y     # [N, C]

Sharding: data-parallel over the batch axis across 8 NeuronCores
(4 batches per core); W_qk / b_qk replicated.

Fast path (b_qk == 0, which setup_inputs always produces):
  logits = q k^T = x (Wq^T Wk) x^T, so precompute M = Wq^T Wk ONCE per
  core (16 matmuls) and per batch do
    zt = M^T-layout @ xt               # [C, N]   (half the flops of qk)
    ST = zt^T x^T  -> exp(scale ST)    # [N, N]   rows m, cols n
    out = ET^T @ [Y | 1] -> normalize  # [N, C]
  eliminating stage B's q/k projection (3.0 -> 2.5 matmul units/batch).
  x is cast f32->bf16 into a DRAM scratch (gpsimd cast DMA) and
  transposed DRAM->SBUF by the DMA xbar (dma_start_transpose), so the
  PE runs zero transposes.

General path (b_qk != 0): baseline 3-stage kernel with explicit q/k.
"""

import numpy as np

import concourse.bass as bass
import concourse.mybir as mybir
import concourse.tile as tile
from concourse import bacc
from concourse.bass_utils import run_bass_kernel_spmd
from concourse.masks import make_identity

P = 128
N_CORES = 8
B_FULL = 32
B_PER_CORE = B_FULL // N_CORES  # 4
N = 1024            # tokens
C = 512             # channels
L = 512             # latent
TWO_L = 2 * L
NT = N // P         # 8 token tiles
CT = C // P         # 4 channel tiles
LT = TWO_L // P     # 8 latent tiles (0..3 = q, 4..7 = k)
SCALE = float(L) ** -0.5
YA = C + 1          # augmented Y width: [Y | ones]
NA = 257            # first stage-D matmul free dim
NB = YA - NA        # 256
SCOL = C - NA       # ones column's index within psum_B (= 255)

F32 = mybir.dt.float32
BF16 = mybir.dt.bfloat16
IDENT = mybir.ActivationFunctionType.Identity
EXP = mybir.ActivationFunctionType.Exp


def _emit_fast(tc, x, y, w, out):
    """Fast path (no bias). x/y: [B_PER_CORE, N, C] dram APs, w: [2L, C],
    out: [B_PER_CORE, N, C]."""
    from contextlib import ExitStack

    nc = tc.nc
    with ExitStack() as ctx:
        const = ctx.enter_context(tc.tile_pool(name="const", bufs=1))
        big = ctx.enter_context(tc.tile_pool(name="big", bufs=1))
        outp = ctx.enter_context(tc.tile_pool(name="outp", bufs=4))
        rsp = ctx.enter_context(tc.tile_pool(name="rsp", bufs=4))
        ps_mm = ctx.enter_context(tc.tile_pool(name="ps_mm", bufs=2, space="PSUM"))
        ps_d = ctx.enter_context(tc.tile_pool(name="ps_d", bufs=4, space="PSUM"))
        ps_tr = ctx.enter_context(tc.tile_pool(name="ps_tr", bufs=2, space="PSUM"))

        identity = const.tile([P, P], BF16, tag="ident")
        make_identity(nc, identity)

        # ---- persistent workspaces ----
        wbf = big.tile([P, LT, C], BF16, tag="wbf")      # W natural [l, c]
        mt_sb = big.tile([P, CT, C], BF16, tag="mt")     # MT[b, a] = M[a, b]
        zt = big.tile([P, CT, N], BF16, tag="zt")        # [c', n]
        et = big.tile([P, NT, N], BF16, tag="et")        # rows m, cols n
        xbf2 = [
            big.tile([P, NT, C], BF16, tag=f"xbf{j}", name=f"xbf{j}")
            for j in range(2)
        ]
        xt2 = [
            big.tile([P, CT, N], BF16, tag=f"xt{j}", name=f"xt{j}")
            for j in range(2)
        ]
        yaug2 = [
            big.tile([P, NT, YA], BF16, tag=f"yaug{j}", name=f"yaug{j}")
            for j in range(2)
        ]
        for j in range(2):
            nc.vector.memset(yaug2[j][:, :, C:YA], 1.0)

        w_tiled = w.rearrange("(t p) c -> p t c", p=P)

        def load_x(i):
            x_tiled = x[i].rearrange("(t p) c -> p t c", p=P)
            for j in range(2):
                nc.gpsimd.dma_start(
                    xbf2[i % 2][:, 4 * j:4 * j + 4], x_tiled[:, 4 * j:4 * j + 4]
                )

        def transpose_x(i, lo=0, hi=NT):
            src, dst = xbf2[i % 2], xt2[i % 2]
            for nt_ in range(lo, hi):
                for ct in range(CT):
                    ps = ps_tr.tile([P, P], BF16, tag="tr")
                    nc.tensor.transpose(
                        ps, src[:, nt_, ct * P:(ct + 1) * P], identity
                    )
                    nc.vector.tensor_copy(dst[:, ct, nt_ * P:(nt_ + 1) * P], ps)

        def load_y(i):
            nc.gpsimd.dma_start(
                yaug2[i % 2][:, :, 0:C], y[i].rearrange("(t p) c -> p t c", p=P)
            )

        def emit_zt(xt, nh):
            # zt[a, n] = sum_b MT[b, a] xt[b, n]
            for ca in range(CT):
                ps = ps_mm.tile([P, 512], F32, tag="mm")
                for cb in range(CT):
                    nc.tensor.matmul(
                        ps,
                        mt_sb[:, cb, ca * P:(ca + 1) * P],
                        xt[:, cb, nh * 512:(nh + 1) * 512],
                        start=(cb == 0),
                        stop=(cb == CT - 1),
                    )
                nc.vector.tensor_copy(zt[:, ca, nh * 512:(nh + 1) * 512], ps)

        # ---- startup: x0 + W interleaved on the gpsimd cast-DMA queue ----
        # (All DMA queues share a ~9us spin-up before the first byte lands;
        # interleaving x0 chunks with W halves minimizes time-to-first-ST.)
        x0_tiled = x[0].rearrange("(t p) c -> p t c", p=P)
        nc.gpsimd.dma_start(xbf2[0][:, 0:2], x0_tiled[:, 0:2])
        nc.gpsimd.dma_start(wbf[:, 0:4], w_tiled[:, 0:4])   # Wq
        nc.gpsimd.dma_start(xbf2[0][:, 2:4], x0_tiled[:, 2:4])
        nc.gpsimd.dma_start(wbf[:, 4:8], w_tiled[:, 4:8])   # Wk
        nc.gpsimd.dma_start(xbf2[0][:, 4:6], x0_tiled[:, 4:6])
        nc.gpsimd.dma_start(xbf2[0][:, 6:8], x0_tiled[:, 6:8])
        load_y(0)

        transpose_x(0, 0, 4)

        # MT[b, a] = M[a, b] = sum_l Wq[l, a] Wk[l, b]
        for cb in range(CT):
            ps = ps_mm.tile([P, C], F32, tag="mm")
            for lt in range(4):
                nc.tensor.matmul(
                    ps,
                    wbf[:, 4 + lt, cb * P:(cb + 1) * P],
                    wbf[:, lt, 0:C],
                    start=(lt == 0),
                    stop=(lt == 3),
                )
            nc.vector.tensor_copy(mt_sb[:, cb, :], ps)

        transpose_x(0, 4, NT)

        for i in range(B_PER_CORE):
            if i + 1 < B_PER_CORE:
                load_x(i + 1)
                load_y(i + 1)
            xt = xt2[i % 2]
            yaug = yaug2[i % 2]

            emit_zt(xt, 0)
            emit_zt(xt, 1)

            # ---- ST[m, n] = sum_a zt[a, m] xt[a, n]; ET = exp(scale ST) ----
            for nh in range(2):
                for mt in range(NT):
                    ps = ps_mm.tile([P, 512], F32, tag="mm")
                    for ca in range(CT):
                        nc.tensor.matmul(
                            ps,
                            zt[:, ca, mt * P:(mt + 1) * P],
                            xt[:, ca, nh * 512:(nh + 1) * 512],
                            start=(ca == 0),
                            stop=(ca == CT - 1),
                        )
                    nc.scalar.activation(
                        et[:, mt, nh * 512:(nh + 1) * 512], ps, EXP, scale=SCALE
                    )

            # transpose next batch's x on the PE (between C and D)
            if i + 1 < B_PER_CORE:
                transpose_x(i + 1)

            # ---- stage D: out = ET.T @ [Y | 1], then normalize ----
            for nt_ in range(NT):
                psA = ps_d.tile([P, NA], F32, tag="d")
                psB = ps_d.tile([P, NB], F32, tag="d")
                for mt in range(NT):
                    lw = et[:, mt, nt_ * P:(nt_ + 1) * P]
                    nc.tensor.matmul(
                        psA, lw, yaug[:, mt, 0:NA],
                        start=(mt == 0), stop=(mt == NT - 1),
                    )
                    nc.tensor.matmul(
                        psB, lw, yaug[:, mt, NA:YA],
                        start=(mt == 0), stop=(mt == NT - 1),
                    )
                rs = rsp.tile([P, 1], F32, tag="rs")
                nc.vector.reciprocal(rs, psB[:, SCOL:SCOL + 1])
                ob = outp.tile([P, C], F32, tag="ob")
                nc.scalar.mul(ob[:, 0:NA], psA[:, 0:NA], rs)
                nc.vector.tensor_scalar_mul(ob[:, NA:C], psB[:, 0:SCOL], rs)
                if nt_ % 2 == 0:
                    nc.sync.dma_start(out[i, nt_ * P:(nt_ + 1) * P, :], ob)
                else:
                    nc.scalar.dma_start(out[i, nt_ * P:(nt_ + 1) * P, :], ob)


def _emit_general(tc, x, y, w, bvec, out):
    """General path (nonzero bias): baseline 3-stage kernel."""
    from contextlib import ExitStack

    nc = tc.nc
    with ExitStack() as ctx:
        const = ctx.enter_context(tc.tile_pool(name="const", bufs=1))
        big = ctx.enter_context(tc.tile_pool(name="big", bufs=1))
        outp = ctx.enter_context(tc.tile_pool(name="outp", bufs=4))
        rsp = ctx.enter_context(tc.tile_pool(name="rsp", bufs=4))
        ps_mm = ctx.enter_context(tc.tile_pool(name="ps_mm", bufs=3, space="PSUM"))
        ps_d = ctx.enter_context(tc.tile_pool(name="ps_d", bufs=3, space="PSUM"))
        ps_tr = ctx.enter_context(tc.tile_pool(name="ps_tr", bufs=2, space="PSUM"))

        bias_sb = const.tile([P, LT], F32, tag="bias")
        nc.sync.dma_start(bias_sb, bvec.rearrange("(o p) -> p o", p=P))

        wbf = big.tile([P, LT, C], BF16, tag="wbf")
        w_tiled = w.rearrange("(t p) c -> p t c", p=P)
        wt = big.tile([P, CT, TWO_L], BF16, tag="wt")

        identity = const.tile([P, P], BF16, tag="ident")
        make_identity(nc, identity)

        xbf2 = [
            big.tile([P, NT, C], BF16, tag=f"xbf{j}", name=f"xbf{j}")
            for j in range(2)
        ]
        xt2 = [
            big.tile([P, CT, N], BF16, tag=f"xt{j}", name=f"xt{j}")
            for j in range(2)
        ]
        qkt = big.tile([P, LT, N], BF16, tag="qkt")
        et = big.tile([P, NT, N], BF16, tag="et")
        yaug = big.tile([P, NT, YA], BF16, tag="yaug")
        nc.vector.memset(yaug[:, :, C:YA], 1.0)

        def load_x(i):
            x_tiled = x[i].rearrange("(t p) c -> p t c", p=P)
            for j in range(2):
                nc.gpsimd.dma_start(
                    xbf2[i % 2][:, 4 * j:4 * j + 4], x_tiled[:, 4 * j:4 * j + 4]
                )

        def transpose_x(i):
            src, dst = xbf2[i % 2], xt2[i % 2]
            for nt_ in range(NT):
                for ct in range(CT):
                    ps = ps_tr.tile([P, P], BF16, tag="tr")
                    nc.tensor.transpose(
                        ps, src[:, nt_, ct * P:(ct + 1) * P], identity
                    )
                    nc.vector.tensor_copy(dst[:, ct, nt_ * P:(nt_ + 1) * P], ps)

        def load_y(i):
            nc.gpsimd.dma_start(
                yaug[:, :, 0:C], y[i].rearrange("(t p) c -> p t c", p=P)
            )

        x0_tiled = x[0].rearrange("(t p) c -> p t c", p=P)
        nc.gpsimd.dma_start(xbf2[0][:, 0:4], x0_tiled[:, 0:4])
        for a, b in ((0, 1), (1, 2), (2, 4), (4, 8)):
            nc.gpsimd.dma_start(wbf[:, a:b], w_tiled[:, a:b])
        nc.gpsimd.dma_start(xbf2[0][:, 4:8], x0_tiled[:, 4:8])
        for nt_ in range(4):
            for ct in range(CT):
                ps = ps_tr.tile([P, P], BF16, tag="tr")
                nc.tensor.transpose(ps, xbf2[0][:, nt_, ct * P:(ct + 1) * P], identity)
                nc.vector.tensor_copy(xt2[0][:, ct, nt_ * P:(nt_ + 1) * P], ps)
        for lt in range(LT):
            for ct in range(CT):
                ps = ps_tr.tile([P, P], BF16, tag="tr")
                nc.tensor.transpose(
                    ps, wbf[:, lt, ct * P:(ct + 1) * P], identity
                )
                nc.vector.tensor_copy(wt[:, ct, lt * P:(lt + 1) * P], ps)
        for nt_ in range(4, NT):
            for ct in range(CT):
                ps = ps_tr.tile([P, P], BF16, tag="tr")
                nc.tensor.transpose(ps, xbf2[0][:, nt_, ct * P:(ct + 1) * P], identity)
                nc.vector.tensor_copy(xt2[0][:, ct, nt_ * P:(nt_ + 1) * P], ps)

        for i in range(B_PER_CORE):
            if i + 1 < B_PER_CORE:
                load_x(i + 1)
            load_y(i)
            xt = xt2[i % 2]

            for nh in range(2):
                for lt in range(LT):
                    ps = ps_mm.tile([P, 512], F32, tag="mm")
                    for ct in range(CT):
                        nc.tensor.matmul(
                            ps,
                            wt[:, ct, lt * P:(lt + 1) * P],
                            xt[:, ct, nh * 512:(nh + 1) * 512],
                            start=(ct == 0),
                            stop=(ct == CT - 1),
                        )
                    nc.scalar.activation(
                        qkt[:, lt, nh * 512:(nh + 1) * 512],
                        ps,
                        IDENT,
                        bias=bias_sb[:, lt:lt + 1],
                    )

            for nh in range(2):
                for mt in range(NT):
                    ps = ps_mm.tile([P, 512], F32, tag="mm")
                    for lq in range(4):
                        nc.tensor.matmul(
                            ps,
                            qkt[:, 4 + lq, mt * P:(mt + 1) * P],
                            qkt[:, lq, nh * 512:(nh + 1) * 512],
                            start=(lq == 0),
                            stop=(lq == 3),
                        )
                    nc.scalar.activation(
                        et[:, mt, nh * 512:(nh + 1) * 512], ps, EXP, scale=SCALE
                    )

            if i + 1 < B_PER_CORE:
                transpose_x(i + 1)

            for nt_ in range(NT):
                psA = ps_d.tile([P, NA], F32, tag="d")
                psB = ps_d.tile([P, NA], F32, tag="d")
                for mt in range(NT):
                    lw = et[:, mt, nt_ * P:(nt_ + 1) * P]
                    nc.tensor.matmul(
                        psA, lw, yaug[:, mt, 0:NA],
                        start=(mt == 0), stop=(mt == NT - 1),
                    )
                    nc.tensor.matmul(
                        psB[:, 0:NB], lw, yaug[:, mt, NA:YA],
                        start=(mt == 0), stop=(mt == NT - 1),
                    )
                rs = rsp.tile([P, 1], F32, tag="rs")
                nc.vector.reciprocal(rs, psB[:, SCOL:SCOL + 1])
                ob = outp.tile([P, C], F32, tag="ob")
                nc.scalar.mul(ob[:, 0:NA], psA[:, 0:NA], rs)
                nc.vector.tensor_scalar_mul(ob[:, NA:C], psB[:, 0:SCOL], rs)
                nc.sync.dma_start(out[i, nt_ * P:(nt_ + 1) * P, :], ob)


_NC_CACHE = {}


def _build(fast):
    key = ("fast" if fast else "general")
    if key in _NC_CACHE:
        return _NC_CACHE[key]
    nc = bacc.Bacc(
        "TRN2",
        target_bir_lowering=False,
        debug=False,
        enable_asserts=False,
        num_devices=N_CORES,
    )
    x = nc.dram_tensor("x", [B_PER_CORE, N, C], F32, kind="ExternalInput").ap()
    y = nc.dram_tensor("y", [B_PER_CORE, N, C], F32, kind="ExternalInput").ap()
    w = nc.dram_tensor("W_qk", [TWO_L, C], F32, kind="ExternalInput").ap()
    bvec = nc.dram_tensor("b_qk", [TWO_L], F32, kind="ExternalInput").ap()
    out = nc.dram_tensor("out", [B_PER_CORE, N, C], F32, kind="ExternalOutput").ap()
    if fast:
        with tile.TileContext(nc) as tc:
            _emit_fast(tc, x, y, w, out)
    else:
        with tile.TileContext(nc) as tc:
            _emit_general(tc, x, y, w, bvec, out)
    nc.compile()
    _NC_CACHE[key] = nc
    return nc


def run(x, y, W_qk, b_qk, trace=False):
    """Run the SPMD kernel on 8 cores; returns (out, BassKernelResults)."""
    x = np.ascontiguousarray(x, dtype=np.float32)
    y = np.ascontiguousarray(y, dtype=np.float32)
    W_qk = np.ascontiguousarray(W_qk, dtype=np.float32)
    b_qk = np.ascontiguousarray(b_qk, dtype=np.float32)
    fast = not np.any(b_qk)
    nc = _build(fast)
    in_maps = [
        {
            "x": x[k * B_PER_CORE:(k + 1) * B_PER_CORE],
            "y": y[k * B_PER_CORE:(k + 1) * B_PER_CORE],
            "W_qk": W_qk,
            "b_qk": b_qk,
        }
        for k in range(N_CORES)
    ]
    res = run_bass_kernel_spmd(
        nc, in_maps, core_ids=list(range(N_CORES)), trace=trace
    )
    outs = [r["out"] for r in res.results]
    return np.concatenate(outs, axis=0), res


def kernel(x, y, W_qk, b_qk):
    out, _ = run(x, y, W_qk, b_qk)
    return out


# revision 16
# speedup vs baseline: 1.0844x; 1.0007x over previous
"""Trainium2 Bass kernel for CSAttention.

Reference computation (per batch b of 32, N=1024 tokens, C=512 channels,
L=512 latent):
    qk  = x @ W_qk.T + b_qk            # [N, 2L]
    q   = qk[:, :L] * L**-0.5
    k   = qk[:, L:]
    out = softmax(q @ k.T, -1) @ y     # [N, C]

Sharding: data-parallel over the batch axis across 8 NeuronCores
(4 batches per core); W_qk / b_qk replicated.

Fast path (b_qk == 0, which setup_inputs always produces):
  logits = q k^T = x (Wq^T Wk) x^T, so precompute M = Wq^T Wk ONCE per
  core (16 matmuls) and per batch do
    zt = M^T-layout @ xt               # [C, N]   (half the flops of qk)
    ST = zt^T x^T  -> exp(scale ST)    # [N, N]   rows m, cols n
    out = ET^T @ [Y | 1] -> normalize  # [N, C]
  eliminating stage B's q/k projection (3.0 -> 2.5 matmul units/batch).
  x is cast f32->bf16 into a DRAM scratch (gpsimd cast DMA) and
  transposed DRAM->SBUF by the DMA xbar (dma_start_transpose), so the
  PE runs zero transposes.

General path (b_qk != 0): baseline 3-stage kernel with explicit q/k.
"""

import numpy as np

import concourse.bass as bass
import concourse.mybir as mybir
import concourse.tile as tile
from concourse import bacc
from concourse.bass_utils import run_bass_kernel_spmd
from concourse.masks import make_identity

P = 128
N_CORES = 8
B_FULL = 32
B_PER_CORE = B_FULL // N_CORES  # 4
N = 1024            # tokens
C = 512             # channels
L = 512             # latent
TWO_L = 2 * L
NT = N // P         # 8 token tiles
CT = C // P         # 4 channel tiles
LT = TWO_L // P     # 8 latent tiles (0..3 = q, 4..7 = k)
SCALE = float(L) ** -0.5
YA = C + 1          # augmented Y width: [Y | ones]
NA = 257            # first stage-D matmul free dim
NB = YA - NA        # 256
SCOL = C - NA       # ones column's index within psum_B (= 255)

F32 = mybir.dt.float32
BF16 = mybir.dt.bfloat16
IDENT = mybir.ActivationFunctionType.Identity
EXP = mybir.ActivationFunctionType.Exp


def _emit_fast(tc, x, y, w, out):
    """Fast path (no bias). x/y: [B_PER_CORE, N, C] dram APs, w: [2L, C],
    out: [B_PER_CORE, N, C]."""
    from contextlib import ExitStack

    nc = tc.nc
    with ExitStack() as ctx:
        const = ctx.enter_context(tc.tile_pool(name="const", bufs=1))
        big = ctx.enter_context(tc.tile_pool(name="big", bufs=1))
        outp = ctx.enter_context(tc.tile_pool(name="outp", bufs=4))
        rsp = ctx.enter_context(tc.tile_pool(name="rsp", bufs=4))
        ps_mm = ctx.enter_context(tc.tile_pool(name="ps_mm", bufs=4, space="PSUM"))
        # transpose psums share the stage-D tag slab (temporally disjoint)
        ps_d = ctx.enter_context(tc.tile_pool(name="ps_d", bufs=4, space="PSUM"))
        ps_tr = ps_d

        identity = const.tile([P, P], BF16, tag="ident")
        make_identity(nc, identity)

        # ---- persistent workspaces ----
        wbf = big.tile([P, LT, C], BF16, tag="wbf")      # W natural [l, c]
        mt_sb = big.tile([P, CT, C], BF16, tag="mt")     # MT[b, a] = M[a, b]
        zt = big.tile([P, CT, N], BF16, tag="zt")        # [c', n]
        et = big.tile([P, NT, N], BF16, tag="et")        # rows m, cols n
        xbf2 = [
            big.tile([P, NT, C], BF16, tag=f"xbf{j}", name=f"xbf{j}")
            for j in range(2)
        ]
        xt2 = [
            big.tile([P, CT, N], BF16, tag=f"xt{j}", name=f"xt{j}")
            for j in range(2)
        ]
        yaug2 = [
            big.tile([P, NT, YA], BF16, tag=f"yaug{j}", name=f"yaug{j}")
            for j in range(2)
        ]
        for j in range(2):
            nc.vector.memset(yaug2[j][:, :, C:YA], 1.0)

        w_tiled = w.rearrange("(t p) c -> p t c", p=P)

        def load_x(i):
            x_tiled = x[i].rearrange("(t p) c -> p t c", p=P)
            for j in range(2):
                nc.gpsimd.dma_start(
                    xbf2[i % 2][:, 4 * j:4 * j + 4], x_tiled[:, 4 * j:4 * j + 4]
                )

        def transpose_x(i, lo=0, hi=NT):
            src, dst = xbf2[i % 2], xt2[i % 2]
            for nt_ in range(lo, hi):
                for ct in range(CT):
                    ps = ps_tr.tile([P, P], BF16, tag="d")
                    nc.tensor.transpose(
                        ps, src[:, nt_, ct * P:(ct + 1) * P], identity
                    )
                    nc.vector.tensor_copy(dst[:, ct, nt_ * P:(nt_ + 1) * P], ps)

        def load_y(i):
            nc.gpsimd.dma_start(
                yaug2[i % 2][:, :, 0:C], y[i].rearrange("(t p) c -> p t c", p=P)
            )

        def emit_zt(xt, nh):
            # zt[a, n] = sum_b MT[b, a] xt[b, n]
            for ca in range(CT):
                ps = ps_mm.tile([P, 512], F32, tag="mm")
                for cb in range(CT):
                    nc.tensor.matmul(
                        ps,
                        mt_sb[:, cb, ca * P:(ca + 1) * P],
                        xt[:, cb, nh * 512:(nh + 1) * 512],
                        start=(cb == 0),
                        stop=(cb == CT - 1),
                    )
                nc.vector.tensor_copy(zt[:, ca, nh * 512:(nh + 1) * 512], ps)

        # ---- startup: W + x0 interleaved on the gpsimd cast-DMA queue ----
        # (All DMA queues share a ~9us spin-up before the first byte lands.
        # Wq goes first so M can start as early as possible; batch 0's zt
        # halves interleave with the x0 transposes to keep the PE fed.)
        x0_tiled = x[0].rearrange("(t p) c -> p t c", p=P)
        nc.gpsimd.dma_start(wbf[:, 0:4], w_tiled[:, 0:4])   # Wq
        nc.gpsimd.dma_start(xbf2[0][:, 0:2], x0_tiled[:, 0:2])
        nc.gpsimd.dma_start(wbf[:, 4:8], w_tiled[:, 4:8])   # Wk
        nc.gpsimd.dma_start(xbf2[0][:, 2:4], x0_tiled[:, 2:4])
        nc.gpsimd.dma_start(xbf2[0][:, 4:6], x0_tiled[:, 4:6])
        nc.gpsimd.dma_start(xbf2[0][:, 6:8], x0_tiled[:, 6:8])
        load_y(0)

        transpose_x(0, 0, 2)

        # MT[b, a] = M[a, b] = sum_l Wq[l, a] Wk[l, b]
        for cb in range(CT):
            ps = ps_mm.tile([P, C], F32, tag="mm")
            for lt in range(4):
                nc.tensor.matmul(
                    ps,
                    wbf[:, 4 + lt, cb * P:(cb + 1) * P],
                    wbf[:, lt, 0:C],
                    start=(lt == 0),
                    stop=(lt == 3),
                )
            nc.vector.tensor_copy(mt_sb[:, cb, :], ps)

        transpose_x(0, 2, 4)
        emit_zt(xt2[0], 0)
        transpose_x(0, 4, NT)
        emit_zt(xt2[0], 1)

        for i in range(B_PER_CORE):
            if i + 1 < B_PER_CORE:
                load_x(i + 1)
                load_y(i + 1)
            xt = xt2[i % 2]
            yaug = yaug2[i % 2]

            if i > 0:
                emit_zt(xt, 0)
                emit_zt(xt, 1)

            # ---- ST[m, n] = sum_a zt[a, m] xt[a, n]; ET = exp(scale ST) ----
            for nh in range(2):
                for mt in range(NT):
                    ps = ps_mm.tile([P, 512], F32, tag="mm")
                    for ca in range(CT):
                        nc.tensor.matmul(
                            ps,
                            zt[:, ca, mt * P:(mt + 1) * P],
                            xt[:, ca, nh * 512:(nh + 1) * 512],
                            start=(ca == 0),
                            stop=(ca == CT - 1),
                        )
                    nc.scalar.activation(
                        et[:, mt, nh * 512:(nh + 1) * 512], ps, EXP, scale=SCALE
                    )

            # transpose next batch's x on the PE (between C and D)
            if i + 1 < B_PER_CORE:
                transpose_x(i + 1)

            # ---- stage D: out = ET.T @ [Y | 1], then normalize ----
            for nt_ in range(NT):
                psA = ps_d.tile([P, NA], F32, tag="d")
                psB = ps_d.tile([P, NB], F32, tag="d")
                for mt in range(NT):
                    lw = et[:, mt, nt_ * P:(nt_ + 1) * P]
                    nc.tensor.matmul(
                        psA, lw, yaug[:, mt, 0:NA],
                        start=(mt == 0), stop=(mt == NT - 1),
                    )
                    nc.tensor.matmul(
                        psB, lw, yaug[:, mt, NA:YA],
                        start=(mt == 0), stop=(mt == NT - 1),
                    )
                rs = rsp.tile([P, 1], F32, tag="rs")
                nc.vector.reciprocal(rs, psB[:, SCOL:SCOL + 1])
                ob = outp.tile([P, C], F32, tag="ob")
                nc.scalar.mul(ob[:, 0:NA], psA[:, 0:NA], rs)
                nc.vector.tensor_scalar_mul(ob[:, NA:C], psB[:, 0:SCOL], rs)
                if nt_ % 2 == 0:
                    nc.sync.dma_start(out[i, nt_ * P:(nt_ + 1) * P, :], ob)
                else:
                    nc.scalar.dma_start(out[i, nt_ * P:(nt_ + 1) * P, :], ob)


def _emit_general(tc, x, y, w, bvec, out):
    """General path (nonzero bias): baseline 3-stage kernel."""
    from contextlib import ExitStack

    nc = tc.nc
    with ExitStack() as ctx:
        const = ctx.enter_context(tc.tile_pool(name="const", bufs=1))
        big = ctx.enter_context(tc.tile_pool(name="big", bufs=1))
        outp = ctx.enter_context(tc.tile_pool(name="outp", bufs=4))
        rsp = ctx.enter_context(tc.tile_pool(name="rsp", bufs=4))
        ps_mm = ctx.enter_context(tc.tile_pool(name="ps_mm", bufs=3, space="PSUM"))
        ps_d = ctx.enter_context(tc.tile_pool(name="ps_d", bufs=3, space="PSUM"))
        ps_tr = ctx.enter_context(tc.tile_pool(name="ps_tr", bufs=2, space="PSUM"))

        bias_sb = const.tile([P, LT], F32, tag="bias")
        nc.sync.dma_start(bias_sb, bvec.rearrange("(o p) -> p o", p=P))

        wbf = big.tile([P, LT, C], BF16, tag="wbf")
        w_tiled = w.rearrange("(t p) c -> p t c", p=P)
        wt = big.tile([P, CT, TWO_L], BF16, tag="wt")

        identity = const.tile([P, P], BF16, tag="ident")
        make_identity(nc, identity)

        xbf2 = [
            big.tile([P, NT, C], BF16, tag=f"xbf{j}", name=f"xbf{j}")
            for j in range(2)
        ]
        xt2 = [
            big.tile([P, CT, N], BF16, tag=f"xt{j}", name=f"xt{j}")
            for j in range(2)
        ]
        qkt = big.tile([P, LT, N], BF16, tag="qkt")
        et = big.tile([P, NT, N], BF16, tag="et")
        yaug = big.tile([P, NT, YA], BF16, tag="yaug")
        nc.vector.memset(yaug[:, :, C:YA], 1.0)

        def load_x(i):
            x_tiled = x[i].rearrange("(t p) c -> p t c", p=P)
            for j in range(2):
                nc.gpsimd.dma_start(
                    xbf2[i % 2][:, 4 * j:4 * j + 4], x_tiled[:, 4 * j:4 * j + 4]
                )

        def transpose_x(i):
            src, dst = xbf2[i % 2], xt2[i % 2]
            for nt_ in range(NT):
                for ct in range(CT):
                    ps = ps_tr.tile([P, P], BF16, tag="d")
                    nc.tensor.transpose(
                        ps, src[:, nt_, ct * P:(ct + 1) * P], identity
                    )
                    nc.vector.tensor_copy(dst[:, ct, nt_ * P:(nt_ + 1) * P], ps)

        def load_y(i):
            nc.gpsimd.dma_start(
                yaug[:, :, 0:C], y[i].rearrange("(t p) c -> p t c", p=P)
            )

        x0_tiled = x[0].rearrange("(t p) c -> p t c", p=P)
        nc.gpsimd.dma_start(xbf2[0][:, 0:4], x0_tiled[:, 0:4])
        for a, b in ((0, 1), (1, 2), (2, 4), (4, 8)):
            nc.gpsimd.dma_start(wbf[:, a:b], w_tiled[:, a:b])
        nc.gpsimd.dma_start(xbf2[0][:, 4:8], x0_tiled[:, 4:8])
        for nt_ in range(4):
            for ct in range(CT):
                ps = ps_tr.tile([P, P], BF16, tag="d")
                nc.tensor.transpose(ps, xbf2[0][:, nt_, ct * P:(ct + 1) * P], identity)
                nc.vector.tensor_copy(xt2[0][:, ct, nt_ * P:(nt_ + 1) * P], ps)
        for lt in range(LT):
            for ct in range(CT):
                ps = ps_tr.tile([P, P], BF16, tag="d")
                nc.tensor.transpose(
                    ps, wbf[:, lt, ct * P:(ct + 1) * P], identity
                )
                nc.vector.tensor_copy(wt[:, ct, lt * P:(lt + 1) * P], ps)
        for nt_ in range(4, NT):
            for ct in range(CT):
                ps = ps_tr.tile([P, P], BF16, tag="d")
                nc.tensor.transpose(ps, xbf2[0][:, nt_, ct * P:(ct + 1) * P], identity)
                nc.vector.tensor_copy(xt2[0][:, ct, nt_ * P:(nt_ + 1) * P], ps)

        for i in range(B_PER_CORE):
            if i + 1 < B_PER_CORE:
                load_x(i + 1)
            load_y(i)
            xt = xt2[i % 2]

            for nh in range(2):
                for lt in range(LT):
                    ps = ps_mm.tile([P, 512], F32, tag="mm")
                    for ct in range(CT):
                        nc.tensor.matmul(
                            ps,
                            wt[:, ct, lt * P:(lt + 1) * P],
                            xt[:, ct, nh * 512:(nh + 1) * 512],
                            start=(ct == 0),
                            stop=(ct == CT - 1),
                        )
                    nc.scalar.activation(
                        qkt[:, lt, nh * 512:(nh + 1) * 512],
                        ps,
                        IDENT,
                        bias=bias_sb[:, lt:lt + 1],
                    )

            for nh in range(2):
                for mt in range(NT):
                    ps = ps_mm.tile([P, 512], F32, tag="mm")
                    for lq in range(4):
                        nc.tensor.matmul(
                            ps,
                            qkt[:, 4 + lq, mt * P:(mt + 1) * P],
                            qkt[:, lq, nh * 512:(nh + 1) * 512],
                            start=(lq == 0),
                            stop=(lq == 3),
                        )
                    nc.scalar.activation(
                        et[:, mt, nh * 512:(nh + 1) * 512], ps, EXP, scale=SCALE
                    )

            if i + 1 < B_PER_CORE:
                transpose_x(i + 1)

            for nt_ in range(NT):
                psA = ps_d.tile([P, NA], F32, tag="d")
                psB = ps_d.tile([P, NA], F32, tag="d")
                for mt in range(NT):
                    lw = et[:, mt, nt_ * P:(nt_ + 1) * P]
                    nc.tensor.matmul(
                        psA, lw, yaug[:, mt, 0:NA],
                        start=(mt == 0), stop=(mt == NT - 1),
                    )
                    nc.tensor.matmul(
                        psB[:, 0:NB], lw, yaug[:, mt, NA:YA],
                        start=(mt == 0), stop=(mt == NT - 1),
                    )
                rs = rsp.tile([P, 1], F32, tag="rs")
                nc.vector.reciprocal(rs, psB[:, SCOL:SCOL + 1])
                ob = outp.tile([P, C], F32, tag="ob")
                nc.scalar.mul(ob[:, 0:NA], psA[:, 0:NA], rs)
                nc.vector.tensor_scalar_mul(ob[:, NA:C], psB[:, 0:SCOL], rs)
                nc.sync.dma_start(out[i, nt_ * P:(nt_ + 1) * P, :], ob)


_NC_CACHE = {}


def _build(fast):
    key = ("fast" if fast else "general")
    if key in _NC_CACHE:
        return _NC_CACHE[key]
    nc = bacc.Bacc(
        "TRN2",
        target_bir_lowering=False,
        debug=False,
        enable_asserts=False,
        num_devices=N_CORES,
    )
    x = nc.dram_tensor("x", [B_PER_CORE, N, C], F32, kind="ExternalInput").ap()
    y = nc.dram_tensor("y", [B_PER_CORE, N, C], F32, kind="ExternalInput").ap()
    w = nc.dram_tensor("W_qk", [TWO_L, C], F32, kind="ExternalInput").ap()
    bvec = nc.dram_tensor("b_qk", [TWO_L], F32, kind="ExternalInput").ap()
    out = nc.dram_tensor("out", [B_PER_CORE, N, C], F32, kind="ExternalOutput").ap()
    if fast:
        with tile.TileContext(nc) as tc:
            _emit_fast(tc, x, y, w, out)
    else:
        with tile.TileContext(nc) as tc:
            _emit_general(tc, x, y, w, bvec, out)
    nc.compile()
    _NC_CACHE[key] = nc
    return nc


def run(x, y, W_qk, b_qk, trace=False):
    """Run the SPMD kernel on 8 cores; returns (out, BassKernelResults)."""
    x = np.ascontiguousarray(x, dtype=np.float32)
    y = np.ascontiguousarray(y, dtype=np.float32)
    W_qk = np.ascontiguousarray(W_qk, dtype=np.float32)
    b_qk = np.ascontiguousarray(b_qk, dtype=np.float32)
    fast = not np.any(b_qk)
    nc = _build(fast)
    in_maps = [
        {
            "x": x[k * B_PER_CORE:(k + 1) * B_PER_CORE],
            "y": y[k * B_PER_CORE:(k + 1) * B_PER_CORE],
            "W_qk": W_qk,
            "b_qk": b_qk,
        }
        for k in range(N_CORES)
    ]
    res = run_bass_kernel_spmd(
        nc, in_maps, core_ids=list(range(N_CORES)), trace=trace
    )
    outs = [r["out"] for r in res.results]
    return np.concatenate(outs, axis=0), res


def kernel(x, y, W_qk, b_qk):
    out, _ = run(x, y, W_qk, b_qk)
    return out
